# revision 28
# baseline (speedup 1.0000x reference)
"""3-layer GCN (GCNConv x3 + leaky_relu + first-node-per-graph readout) on
8 Trainium2 NeuronCores via Bass/Tile.

Only the 100 first-node rows of layer 3 are read out, so the network is
sliced backward on the host:
  L3 edges: dst is a first node            (~1.6k edges)
  S2 = srcs of L3 edges                    (~1.6k nodes needing h2/z)
  L2 edges: dst in S2                      (~26k edges)
  S1 = srcs of L2 edges                    (~14k nodes needing h1)
  L1 edges: dst in S1                      (~230k edges)

Device strategy (one collective total):
  - x is pre-cast to bf16 and replicated to every core; L1 messages are
    dma_gathered per edge directly from it (no stage-A, no x AllGather).
  - The GCN normalization dis[src]*dis[dst] is folded into the one-hot
    scatter matrices S (built on DVE via iota==slot fused with *norm), so
    aggregation is a plain PE matmul accumulation and no dis scaling
    remains elsewhere.  Bias is injected with a rank-1 PE matmul
    (lhsT row-0-ones, rhs row-0=b); leaky_relu runs on the Scalar engine.
  - L1 dst nodes are bin-packed into windows of 128 with balanced edge
    counts so every (core, window) runs the same chunk count (SPMD).
  - h1 rows are exchanged with a single AllToAll of per-(owner,consumer)
    edge messages (~2MB) instead of AllGathering the 10MB h1 table.
  - L2/L3 are partitioned by CONSUMER (owner of the destination graph), so
    z never leaves the core: L3 gathers z locally and writes the output.
  - dma_gather is descriptor-latency-bound (~8.6 ns/idx on one SWDGE
    queue); alternating gathers across two SWDGE queues
    (num_swdge_queues=2, queue_num=b%2) runs ~6x faster.

kernel(**inputs) takes the full unsharded inputs and returns the full
[n_graphs, 32] float32 output.
"""

import sys

sys.path.insert(0, "/opt/trn_rl_repo")

import numpy as np

import concourse.bacc as bacc
import concourse.mybir as mybir
import concourse.tile as tile
from concourse.bass_utils import run_bass_kernel_spmd

F32 = mybir.dt.float32
BF16 = mybir.dt.bfloat16
I16 = mybir.dt.int16
NPBF16 = mybir.dt.np(BF16)

N_CORES = 8
C0, C1, C2, C3 = 128, 256, 256, 32
ZPAD = 64  # z-table row padded to 64 f32 (256B dma_gather granularity)

# ---------------------------------------------------------------------------
# Host-side prep
# ---------------------------------------------------------------------------


def _pack_gather_idx(idx, n_slots):
    """int32 row indices -> dma_gather int16 layout [128, n_slots//16]."""
    assert n_slots % 16 == 0
    a = np.zeros(n_slots, np.int16)
    a[: len(idx)] = idx.astype(np.int16)
    a = a.reshape(n_slots // 16, 16).T
    return np.tile(a, (8, 1))


def _pack_chunked(vals, n_slots, fill, dtype=NPBF16):
    """values per edge -> [128, n_slots//128] (edge j at [j%128, j//128])."""
    a = np.full(n_slots, fill, np.float32)
    a[: len(vals)] = vals
    return a.reshape(n_slots // 128, 128).T.astype(dtype).copy()


def _ffd_pack(nodes, weights, n_bins, cap_nodes, cap_weight):
    """First-fit-decreasing: assign nodes to bins; returns row id per node
    (bin*128 + index) or None if infeasible."""
    order = np.argsort(-weights, kind="stable")
    bin_w = np.zeros(n_bins, np.int64)
    bin_n = np.zeros(n_bins, np.int64)
    row = np.zeros(len(nodes), np.int64)
    for k in order:
        w = weights[k]
        placed = False
        for b in range(n_bins):
            if bin_n[b] < cap_nodes and bin_w[b] + w <= cap_weight:
                row[k] = b * 128 + bin_n[b]
                bin_n[b] += 1
                bin_w[b] += w
                placed = True
                break
        if not placed:
            return None
    return row


def host_prep(x, src, dst, batch, W1, b1, W2, b2, W3, b3, n_graphs):
    N = x.shape[0]
    G = int(n_graphs)
    E = len(src)
    NPC = (N + N_CORES - 1) // N_CORES
    src = np.asarray(src).astype(np.int64)
    dst = np.asarray(dst).astype(np.int64)

    deg = np.bincount(dst, minlength=N).astype(np.float64)
    dis = np.where(deg > 0, 1.0 / np.sqrt(np.maximum(deg, 1.0)), 0.0)

    first = np.full(G, N, np.int64)
    np.minimum.at(first, np.asarray(batch).astype(np.int64), np.arange(N))
    gowner = first // NPC
    graphs_per_core = [np.nonzero(gowner == c)[0] for c in range(N_CORES)]
    gslot = np.full(G, -1, np.int64)
    for c in range(N_CORES):
        gslot[graphs_per_core[c]] = np.arange(len(graphs_per_core[c]))

    is_first = np.zeros(N, bool)
    is_first[first] = True
    gid_of_first = np.full(N, -1, np.int64)
    gid_of_first[first] = np.arange(G)

    # ---- L3 edges and S2 per consumer core ----
    e3 = np.nonzero(is_first[dst])[0]
    e3_consumer = gowner[gid_of_first[dst[e3]]]
    S2 = []          # per core: node ids (zslot order)
    zslot_of = []    # per core: dict node -> zslot
    e3_by_core = []
    for c in range(N_CORES):
        ec = e3[e3_consumer == c]
        e3_by_core.append(ec)
        uniq = np.unique(src[ec]) if len(ec) else np.zeros(0, np.int64)
        S2.append(uniq)
        zslot_of.append({int(n): i for i, n in enumerate(uniq)})
    S2COLS = max(128, int(-(-max((len(u) for u in S2), default=1) // 128)) * 128)
    NCH3 = max(1, int(-(-max((len(ec) for ec in e3_by_core), default=1) // 128)))

    # ---- L2 edges per consumer core ----
    e2_by_core = []
    for c in range(N_CORES):
        m = np.zeros(N, bool)
        m[S2[c]] = True
        e2_by_core.append(np.nonzero(m[dst])[0])

    # owner block sizes -> BE
    cnt_ic = np.zeros((N_CORES, N_CORES), np.int64)
    for c in range(N_CORES):
        ow = src[e2_by_core[c]] // NPC
        cnt_ic[:, c] = np.bincount(ow, minlength=N_CORES)
    BE = int(-(-max(1, int(cnt_ic.max())) // 128)) * 128
    NSL2 = N_CORES * BE          # a2a slots per core
    NCH2 = NSL2 // 128

    # ---- S1 and per-core L1 structure ----
    all_l2_src = np.concatenate([src[e2_by_core[c]] for c in range(N_CORES)])
    S1 = np.unique(all_l2_src) if len(all_l2_src) else np.zeros(0, np.int64)
    in_S1 = np.zeros(N, bool)
    in_S1[S1] = True
    e1 = np.nonzero(in_S1[dst])[0]
    e1_home = dst[e1] // NPC

    S1_by_core = [S1[(S1 >= c * NPC) & (S1 < (c + 1) * NPC)] for c in range(N_CORES)]
    maxn = max((len(s) for s in S1_by_core), default=1)
    Wmin = max(1, -(-maxn // 128))
    maxe = max(
        (int((e1_home == c).sum()) for c in range(N_CORES)), default=1
    )

    def try_fit(W_, P_):
        ok = []
        for c in range(N_CORES):
            nodes = S1_by_core[c]
            r = _ffd_pack(nodes, deg[nodes], W_, 128, P_ * 128)
            if r is None:
                return None
            ok.append(r)
        return ok

    # pick feasible (W, P) minimizing total chunk count W*P
    best = None
    for W_ in range(Wmin, Wmin + 5):
        Pmin = max(1, -(-maxe // (128 * W_)))
        for P_ in range(Pmin, Pmin + 8):
            if best is not None and W_ * P_ >= best[0] * best[1]:
                break
            rows = try_fit(W_, P_)
            if rows is not None:
                best = (W_, P_, rows)
                break
    if best is None:
        raise RuntimeError("L1 window packing failed")
    W, P, rows_by_core = best
    NCH1 = W * P
    H1R = W * 128

    # local h1 row per global node (per core)
    h1row = np.full(N, -1, np.int64)
    for c in range(N_CORES):
        h1row[S1_by_core[c]] = rows_by_core[c]

    in_maps = []
    meta_rows = []
    W2blk = np.zeros((128, 512), np.float32)
    for k in range(2):
        for h in range(2):
            W2blk[:, (k * 2 + h) * 128 : (k * 2 + h + 1) * 128] = W2[
                k * 128 : (k + 1) * 128, h * 128 : (h + 1) * 128
            ]
    W3blk = np.zeros((128, 2 * C3), np.float32)
    for k in range(2):
        W3blk[:, k * C3 : (k + 1) * C3] = W3[k * 128 : (k + 1) * 128, :]
    E0 = np.zeros((128, 128), np.float32)
    E0[0, :] = 1.0
    b1row = np.zeros((128, C1), np.float32)
    b1row[0, :] = b1
    b2c = np.stack([b2[0:128], b2[128:256]], axis=1)  # [128, 2]
    b3row = np.zeros((128, ZPAD), np.float32)
    b3row[0, :C3] = b3
    iota128 = np.tile(np.arange(128, dtype=np.float32)[None, :], (128, 1))
    iota256 = np.tile(np.arange(S2COLS, dtype=np.float32)[None, :], (128, 1))

    xbf = np.ascontiguousarray(x).astype(NPBF16)

    for c in range(N_CORES):
        # ---- L1 tables ----
        ec = e1[e1_home == c]
        win = h1row[dst[ec]] // 128
        slot = h1row[dst[ec]] % 128
        order = np.argsort(win, kind="stable")
        ec, win, slot = ec[order], win[order], slot[order]
        idx1 = np.zeros(NCH1 * 128, np.int64)
        sl1 = np.full(NCH1 * 128, -1.0, np.float64)
        nm1 = np.zeros(NCH1 * 128, np.float64)
        ptr = np.searchsorted(win, np.arange(W + 1))
        for w in range(W):
            ee = ec[ptr[w] : ptr[w + 1]]
            k0 = w * P * 128
            n = len(ee)
            assert n <= P * 128
            idx1[k0 : k0 + n] = src[ee]
            sl1[k0 : k0 + n] = slot[ptr[w] : ptr[w + 1]]
            nm1[k0 : k0 + n] = dis[src[ee]] * dis[dst[ee]]

        # ---- L2 consumer tables + a2a slot assignment ----
        ec2 = e2_by_core[c]
        ow = src[ec2] // NPC
        order2 = np.argsort(ow, kind="stable")
        ec2, ow = ec2[order2], ow[order2]
        pos = np.arange(len(ec2)) - np.searchsorted(ow, ow)  # pos within owner
        slot2 = ow * BE + pos
        zs = np.array([zslot_of[c][int(n)] for n in dst[ec2]], np.int64) if len(ec2) else np.zeros(0, np.int64)
        sl2 = np.full(NSL2, -1.0, np.float64)
        nm2 = np.zeros(NSL2, np.float64)
        sl2[slot2] = zs
        nm2[slot2] = dis[src[ec2]] * dis[dst[ec2]]

        # ---- L3: dense multi-hot aggregation matrix M[s, g] = sum(norm) ----
        Mw = np.zeros((S2COLS // 128, 128, 128), np.float64)
        ec3 = e3_by_core[c]
        for e in ec3:
            s = zslot_of[c][int(src[e])]
            g = gslot[gid_of_first[dst[e]]]
            Mw[s // 128, s % 128, g] += dis[src[e]] * dis[dst[e]]
        Mpack = np.concatenate(list(Mw), axis=1).astype(np.float32)

        in_maps.append(
            {
                "xbf": xbf,
                "idx1": _pack_gather_idx(idx1, NCH1 * 128),
                "slotv1": _pack_chunked(sl1, NCH1 * 128, -1.0, np.float32),
                "normv1": _pack_chunked(nm1, NCH1 * 128, 0.0, np.float32),
                "slotv2": _pack_chunked(sl2, NSL2, -1.0, np.float32),
                "normv2": _pack_chunked(nm2, NSL2, 0.0, np.float32),
                "m3": Mpack,
                "w1": W1.astype(NPBF16),
                "w2blk": W2blk.astype(NPBF16),
                "w3blk": W3blk.astype(NPBF16),
                "e0": E0.astype(NPBF16),
                "e0f": E0,
                "b1row": b1row.astype(NPBF16),
                "b2c": b2c.astype(np.float32),
                "b3row": b3row,
                "iota128": iota128.astype(NPBF16),
                "iota128f": iota128.astype(np.float32),
                "iota256": iota256.astype(NPBF16),
            }
        )
        meta_rows.append(None)

    # ---- sender-side gather index tables (needs h1row of remote srcs) ----
    # send slot on owner i for consumer c at position pos: c*BE + pos,
    # value = local h1 row on core i of the edge's src.
    sidx = [np.zeros(NSL2, np.int64) for _ in range(N_CORES)]
    for c in range(N_CORES):
        ec2 = e2_by_core[c]
        ow = src[ec2] // NPC
        order2 = np.argsort(ow, kind="stable")
        ec2, ow = ec2[order2], ow[order2]
        pos = np.arange(len(ec2)) - np.searchsorted(ow, ow)
        rows = h1row[src[ec2]]
        assert (rows >= 0).all()
        for i in range(N_CORES):
            m = ow == i
            sidx[i][c * BE + pos[m]] = rows[m]
    for c in range(N_CORES):
        in_maps[c]["sidx"] = _pack_gather_idx(sidx[c], NSL2)

    # pack the many small tables into one param per dtype (3 DMAs at start
    # instead of ~20 serial ones)
    I16_KEYS = ["idx1", "sidx"]
    F32_KEYS = ["slotv1", "normv1", "slotv2", "normv2",
                "e0f", "b2c", "b3row", "m3"]
    B16_KEYS = ["w1", "w2blk", "w3blk", "e0", "b1row", "iota128", "iota256"]
    for c in range(N_CORES):
        m = in_maps[c]
        packed = {"xbf": m["xbf"]}
        packed["ci16"] = np.concatenate([m[k] for k in I16_KEYS], axis=1)
        packed["cf32"] = np.concatenate(
            [np.ascontiguousarray(m[k], dtype=np.float32) for k in F32_KEYS],
            axis=1)
        packed["cb16"] = np.concatenate(
            [np.ascontiguousarray(m[k]).astype(NPBF16) for k in B16_KEYS],
            axis=1)
        in_maps[c] = packed

    meta = dict(
        N=N, G=G, W=W, P=P, NCH1=NCH1, H1R=H1R, BE=BE, NSL2=NSL2,
        NCH2=NCH2, S2COLS=S2COLS, NCH3=NCH3,
        graphs_per_core=graphs_per_core,
    )
    return in_maps, meta


# ---------------------------------------------------------------------------
# Device program
# ---------------------------------------------------------------------------


def build_program(meta, compile_=True, repeat=1, sim=False):
    W, P, NCH1 = meta["W"], meta["P"], meta["NCH1"]
    H1R, NSL2, NCH2 = meta["H1R"], meta["NSL2"], meta["NCH2"]
    S2COLS, NCH3 = meta["S2COLS"], meta["NCH3"]
    NW2 = S2COLS // 128
    N = meta["N"]

    nc = bacc.Bacc(
        "TRN2", target_bir_lowering=False, debug=False,
        num_devices=1 if sim else N_CORES,
        num_swdge_queues=2,
    )
    dp = nc.declare_dram_parameter
    # packed const layouts (must match host_prep's I16/F32/B16_KEYS order)
    ICOLS = NCH1 * 8 + NSL2 // 16
    I_IDX1, I_SIDX = 0, NCH1 * 8
    F_SL1, F_NM1 = 0, NCH1
    F_SL2, F_NM2 = 2 * NCH1, 2 * NCH1 + NCH2
    F_E0F = 2 * NCH1 + 2 * NCH2
    F_B2C = F_E0F + 128
    F_B3R = F_B2C + 2
    F_M3 = F_B3R + ZPAD
    FCOLS = F_M3 + S2COLS
    B_W1, B_W2, B_W3 = 0, C1, C1 + 512
    B_E0 = B_W3 + 2 * C3
    B_B1R = B_E0 + 128
    B_IOTA = B_B1R + C1
    B_IOTA2 = B_IOTA + 128
    BCOLS = B_IOTA2 + S2COLS
    xbf_d = dp("xbf", [N, C0], BF16, isOutput=False)
    ci16_d = dp("ci16", [128, ICOLS], I16, isOutput=False)
    cf32_d = dp("cf32", [128, FCOLS], F32, isOutput=False)
    cb16_d = dp("cb16", [128, BCOLS], BF16, isOutput=False)
    out_d = dp("out", [128, ZPAD], F32, isOutput=True)

    rg = [list(range(N_CORES))]
    AL = mybir.AluOpType
    ACTF = mybir.ActivationFunctionType

    with tile.TileContext(nc) as tc:
        with (
            tc.tile_pool(name="const", bufs=1) as cpool,
            tc.tile_pool(name="work", bufs=4) as pool,
            tc.tile_pool(name="gath", bufs=6) as gpool,
            tc.tile_pool(name="big", bufs=1) as bigpool,
            tc.tile_pool(name="psum", bufs=2, space="PSUM") as psum,
            tc.tile_pool(name="psum1", bufs=1, space="PSUM") as psum1,
            tc.tile_pool(name="dram", bufs=1, space="DRAM") as dram,
        ):
            def ld(d, shape, dt):
                t = cpool.tile(shape, dt, name=d.name + "_sb",
                               tag=d.name + "_sb")
                nc.sync.dma_start(out=t[:], in_=d[:, :])
                return t

            ci16 = ld(ci16_d, [128, ICOLS], I16)
            cf32 = ld(cf32_d, [128, FCOLS], F32)
            cb16 = ld(cb16_d, [128, BCOLS], BF16)

            for _rep in range(repeat):
                h1_tab = dram.tile([H1R, C1], BF16)
                a2a_in = dram.tile([NSL2, C1], BF16)
                a2a_out = dram.tile([NSL2, C1], BF16)

                # ---------------- L1 ----------------
                NB1 = -(-NCH1 // 8)
                gtiles = []
                for b in range(NB1):
                    cs = min(8, NCH1 - b * 8)
                    g = gpool.tile([128, 8, C0], BF16, tag="g1")
                    nc.gpsimd.dma_gather(
                        g[:, 0:cs, :], xbf_d[:, :],
                        ci16[:, I_IDX1 + b * 64 : I_IDX1 + b * 64 + cs * 8],
                        num_idxs=cs * 128, num_idxs_reg=cs * 128,
                        elem_size=C0, queue_num=b % 2,
                    )
                    gtiles.append(g)

                for w in range(W):
                    aggp = psum.tile([128, 128], F32, tag="agg")
                    for ci in range(P):
                        k = w * P + ci
                        S = pool.tile([128, 128], BF16, tag="S1")
                        nc.vector.tensor_scalar(
                            S[:], cb16[:, B_IOTA : B_IOTA + 128],
                            cf32[:, F_SL1 + k : F_SL1 + k + 1], cf32[:, F_NM1 + k : F_NM1 + k + 1],
                            AL.is_equal, AL.mult,
                        )
                        nc.tensor.matmul(
                            aggp[:],
                            lhsT=gtiles[k // 8][:, k % 8, :],
                            rhs=S[:],
                            start=(ci == 0), stop=(ci == P - 1),
                        )
                    agg = pool.tile([128, 128], BF16, tag="aggsb")
                    nc.scalar.activation(agg[:], aggp[:], ACTF.Copy)
                    h1p = psum.tile([128, C1], F32, tag="h1p")
                    nc.tensor.matmul(
                        h1p[:], lhsT=cb16[:, B_E0 : B_E0 + 128], rhs=cb16[:, B_B1R : B_B1R + C1], start=True, stop=False
                    )
                    nc.tensor.matmul(
                        h1p[:], lhsT=agg[:], rhs=cb16[:, B_W1 : B_W1 + C1], start=False, stop=True
                    )
                    h1t = pool.tile([128, C1], BF16, tag="h1t")
                    nc.scalar.activation(h1t[:], h1p[:], ACTF.Lrelu, alpha=0.01)
                    nc.sync.dma_start(
                        out=h1_tab[w * 128 : (w + 1) * 128, :], in_=h1t[:]
                    )

                # ---------------- send build + AllToAll ----------------
                NBS = NSL2 // 1024
                for b in range(NBS):
                    sg = gpool.tile([128, 8, C1], BF16, tag="sg")
                    nc.gpsimd.dma_gather(
                        sg[:, :, :], h1_tab[:, :],
                        ci16[:, I_SIDX + b * 64 : I_SIDX + (b + 1) * 64],
                        num_idxs=1024, num_idxs_reg=1024, elem_size=C1,
                        queue_num=b % 2,
                    )
                    nc.sync.dma_start(
                        out=a2a_in[b * 1024 : (b + 1) * 1024, :].rearrange(
                            "(b a) c -> a b c", a=128
                        ),
                        in_=sg[:, :, :],
                    )
                if sim:
                    nc.sync.dma_start(
                        out=a2a_out[:, :].rearrange("(b a) c -> a b c", a=128),
                        in_=a2a_in[:, :].rearrange("(b a) c -> a b c", a=128),
                    )
                else:
                    nc.gpsimd.collective_compute(
                        "AllToAll", AL.bypass, replica_groups=rg,
                        ins=[a2a_in.opt()], outs=[a2a_out.opt()],
                    )

                # ---------------- L2 ----------------
                QC = NCH2 // 4
                g2t = []
                for q in range(4):
                    g2q = bigpool.tile([128, QC, C1], BF16, tag=f"g2_{q}",
                                       name=f"g2_{q}")
                    nc.sync.dma_start(
                        out=g2q[:, :, :],
                        in_=a2a_out[q * QC * 128 : (q + 1) * QC * 128, :]
                        .rearrange("(b a) c -> a b c", a=128),
                    )
                    g2t.append(g2q)
                agg2p = [
                    psum1.tile([128, S2COLS], F32, tag=f"agg2_{h}",
                               name=f"agg2p_{h}")
                    for h in range(2)
                ]
                for ci in range(NCH2):
                    S2t = pool.tile([128, S2COLS], BF16, tag="S2")
                    nc.vector.tensor_scalar(
                        S2t[:], cb16[:, B_IOTA2 : B_IOTA2 + S2COLS],
                        cf32[:, F_SL2 + ci : F_SL2 + ci + 1], cf32[:, F_NM2 + ci : F_NM2 + ci + 1],
                        AL.is_equal, AL.mult,
                    )
                    for h in range(2):
                        nc.tensor.matmul(
                            agg2p[h][:],
                            lhsT=g2t[ci // QC][:, ci % QC, h * 128 : (h + 1) * 128],
                            rhs=S2t[:],
                            start=(ci == 0), stop=(ci == NCH2 - 1),
                        )
                agg2 = [
                    pool.tile([128, S2COLS], BF16, tag=f"agg2sb_{h}",
                              name=f"agg2sb_{h}")
                    for h in range(2)
                ]
                for h in range(2):
                    nc.scalar.activation(agg2[h][:], agg2p[h][:], ACTF.Copy)

                ztiles = []
                for w2 in range(NW2):
                    h2t = []
                    for h in range(2):
                        h2p = psum1.tile([128, 128], F32, tag="h2p")
                        for k in range(2):
                            nc.tensor.matmul(
                                h2p[:],
                                lhsT=cb16[:, B_W2 + (k * 2 + h) * 128 : B_W2 + (k * 2 + h + 1) * 128],
                                rhs=agg2[k][:, w2 * 128 : (w2 + 1) * 128],
                                start=(k == 0), stop=(k == 1),
                            )
                        t = pool.tile([128, 128], BF16, tag=f"h2t_{h}")
                        nc.scalar.activation(
                            t[:], h2p[:], ACTF.Lrelu,
                            bias=cf32[:, F_B2C + h : F_B2C + h + 1], alpha=0.01,
                        )
                        h2t.append(t)
                    zp = psum1.tile([128, ZPAD], F32, tag="zp")
                    for k in range(2):
                        nc.tensor.matmul(
                            zp[:, 0:C3],
                            lhsT=h2t[k][:],
                            rhs=cb16[:, B_W3 + k * C3 : B_W3 + (k + 1) * C3],
                            start=(k == 0), stop=(k == 1),
                        )
                    zt = pool.tile([128, C3], F32, tag="zt")
                    nc.scalar.activation(zt[:], zp[:, 0:C3], ACTF.Copy)
                    ztiles.append(zt)

                # ---- L3: out[g,:] = b3 + sum_s M[s,g] * z[s,:]  (no gather,
                # z stays in SBUF; M is host-precomputed multi-hot) ----
                op = psum1.tile([128, ZPAD], F32, tag="zp", name="op")
                nc.tensor.matmul(
                    op[:], lhsT=cf32[:, F_E0F : F_E0F + 128],
                    rhs=cf32[:, F_B3R : F_B3R + ZPAD], start=True, stop=False
                )
                for w2 in range(NW2):
                    nc.tensor.matmul(
                        op[:, 0:C3],
                        lhsT=cf32[:, F_M3 + w2 * 128 : F_M3 + (w2 + 1) * 128],
                        rhs=ztiles[w2][:],
                        start=False, stop=(w2 == NW2 - 1),
                    )
                outt = pool.tile([128, ZPAD], F32, tag="outt")
                nc.scalar.activation(outt[:], op[:], ACTF.Copy)
                nc.sync.dma_start(out=out_d[:, :], in_=outt[:])

    if compile_:
        nc.compile()
    return nc


# ---------------------------------------------------------------------------
# Entry point
# ---------------------------------------------------------------------------

_cache = {}


def _prepare(inputs):
    in_maps, meta = host_prep(**inputs)
    key = (meta["W"], meta["P"], meta["BE"], meta["S2COLS"], meta["NCH3"])
    if key not in _cache:
        _cache[key] = build_program(meta)
    return _cache[key], in_maps, meta


def assemble_output(results, meta):
    G = meta["G"]
    out = np.zeros((G, C3), np.float32)
    for i in range(N_CORES):
        gl = meta["graphs_per_core"][i]
        if len(gl):
            out[gl] = results[i]["out"][: len(gl), :C3]
    return out


def kernel(**inputs):
    nc, in_maps, meta = _prepare(inputs)
    res = run_bass_kernel_spmd(nc, in_maps, core_ids=list(range(N_CORES)))
    return assemble_output(res.results, meta)


if __name__ == "__main__":
    rng = np.random.default_rng(0)
    N, E, G = 20000, 320000, 100
    inputs = dict(
        x=rng.standard_normal((N, 128), dtype=np.float32),
        src=rng.integers(0, N, E).astype(np.int32),
        dst=rng.integers(0, N, E).astype(np.int32),
        batch=(np.arange(N) // (N // G)).astype(np.int32),
        W1=rng.standard_normal((128, 256), dtype=np.float32) / 11.3,
        b1=rng.standard_normal(256).astype(np.float32) * 0.01,
        W2=rng.standard_normal((256, 256), dtype=np.float32) / 16.0,
        b2=rng.standard_normal(256).astype(np.float32) * 0.01,
        W3=rng.standard_normal((256, 32), dtype=np.float32) / 16.0,
        b3=rng.standard_normal(32).astype(np.float32) * 0.01,
        n_graphs=G,
    )
    out = kernel(**inputs)
    print("out", out.shape, out.dtype, float(np.abs(out).max()))
